# revision 7
# baseline (speedup 1.0000x reference)
"""Multi-head self-attention (B=4, L=2048, D=1024, H=16, causal + key padding
mask) on 8 Trainium2 NeuronCores.

Sharding: data-parallel over the 4 batches x 2-way split of the query
sequence per batch. The query split is asymmetric ([0:1408) / [1408:2048))
so the causal-attention FLOPs balance: the early-query core needs K/V only
for keys [0:1408) while the late-query core computes K/V for the full
sequence. No cross-core communication; outputs are disjoint row ranges and
the host concatenates them.

Per-core kernel (PE matmuls in fp32r except attention-weights @ V in bf16):
  K^T[d, key] = Wk^T.T @ x^T (+bk per partition), kept in SBUF.
  Q^T likewise, spilled to DRAM and re-read per query block.
  V~[key, h, 0:65] = x^T.T @ Wv^T (+bv) with a ones column per head, so the
  attention @ V matmul also emits the softmax denominator for free.
  Scores are computed transposed, S^T[k, q] = K^T.T @ Q^T, so softmax
  normalization is per-partition and P^T is born in the layout the AV
  matmul wants (zero transposes in the hot path). exp on ScalarE with
  scale=1/8 and the key-padding mask folded into the per-partition bias;
  causal mask = a 0/1 lower-tri multiply on the one diagonal 128-block;
  fully-masked key blocks are skipped outright.
  O~^T[0:65, q] accumulates V~.T @ P^T; row 64 is the denominator.
  Normalize via reciprocal + gpsimd partition-broadcast + multiply into
  y^T (exactly the stationary layout the output projection needs),
  spill y^T to DRAM, then out = y^T.T @ Wo^T + bo.
"""
import math
import threading

import numpy as np
import ml_dtypes

import jax
from jax.sharding import Mesh, PartitionSpec

try:  # jax>=0.8 moved shard_map
    from jax import shard_map
except ImportError:
    from jax.experimental.shard_map import shard_map

import concourse.bass as bass
import concourse.mybir as mybir
import concourse.tile as tile
from concourse import bacc, bass2jax

F32 = mybir.dt.float32
F32R = mybir.dt.float32r
BF16 = mybir.dt.bfloat16
Act = mybir.ActivationFunctionType
Alu = mybir.AluOpType

D_MODEL = 1024
N_HEADS = 16
D_HEAD = 64
B = 4
L = 2048
CUT = 1408  # query split point: balances causal FLOPs across the 2 roles
NCH = D_MODEL // 128


def _r(ap):
    return ap.bitcast(F32R)


def _chunks(lo, hi, step=None):
    """Split [lo, hi) into chunks of width in [256, 512] (>=256 keeps the
    fp32r fast path, <=512 fits one PSUM bank). Prefers 512-wide chunks;
    falls back to 384 when the 512 tail would be <256."""
    if step is None:
        tail = (hi - lo) % 512
        step = 512 if (tail == 0 or tail >= 256) else 384
    out = []
    while lo < hi:
        w = min(step, hi - lo)
        out.append((lo, w))
        lo += w
    assert all(256 <= w <= 512 for _, w in out), out
    return out


def build_role(q0, q1, kmax):
    """Emit the bass program for one role (query rows [q0:q1), keys [0:kmax))."""
    nq = q1 - q0
    nkb_all = kmax // 128
    qbs = _chunks(q0, q1)
    kcs = _chunks(0, kmax)

    nc = bacc.Bacc(None)
    xt = nc.dram_tensor("xt", [D_MODEL, kmax], F32R, kind="ExternalInput")
    wkt = nc.dram_tensor("wkt", [D_MODEL, D_MODEL], F32R, kind="ExternalInput")
    wqt = nc.dram_tensor("wqt", [D_MODEL, D_MODEL], F32R, kind="ExternalInput")
    wvt = nc.dram_tensor("wvt", [D_MODEL, D_MODEL], F32R, kind="ExternalInput")
    wot = nc.dram_tensor("wot", [D_MODEL, D_MODEL], F32R, kind="ExternalInput")
    bk2 = nc.dram_tensor("bk2", [128, NCH], F32, kind="ExternalInput")
    bq2 = nc.dram_tensor("bq2", [128, NCH], F32, kind="ExternalInput")
    bvr = nc.dram_tensor("bvr", [128, D_MODEL], F32, kind="ExternalInput")
    bor = nc.dram_tensor("bor", [128, D_MODEL], F32, kind="ExternalInput")
    tri = nc.dram_tensor("tri", [128, 128], BF16, kind="ExternalInput")
    kbias = nc.dram_tensor("kbias", [128, 16], F32, kind="ExternalInput")
    out = nc.dram_tensor("out", [nq, D_MODEL], F32, kind="ExternalOutput")
    qdram = nc.dram_tensor("qdram", [D_MODEL, nq], F32R)
    ydram = nc.dram_tensor("ydram", [D_MODEL, nq], F32R)

    xt_c = xt.rearrange("(c p) k -> c p k", p=128)
    w_c = {n: t.rearrange("(c p) m -> c p m", p=128)
           for n, t in [("k", wkt), ("q", wqt), ("v", wvt), ("o", wot)]}

    with tile.TileContext(nc) as tc:
        with (
            tc.tile_pool(name="consts", bufs=1) as cst,
            tc.tile_pool(name="stage", bufs=3) as stg,
        ):
            # ---- constants ----
            tri_t = cst.tile([128, 128], BF16, tag="tri")
            nc.sync.dma_start(out=tri_t, in_=tri[:, :])
            kbias_t = cst.tile([128, 16], F32, tag="kbias")
            nc.sync.dma_start(out=kbias_t, in_=kbias[:, :])
            bk_t = cst.tile([128, NCH], F32, tag="bk")
            nc.sync.dma_start(out=bk_t, in_=bk2[:, :])
            bq_t = cst.tile([128, NCH], F32, tag="bq")
            nc.sync.dma_start(out=bq_t, in_=bq2[:, :])
            bvr_t = cst.tile([128, D_MODEL], F32, tag="bvr")
            nc.sync.dma_start(out=bvr_t, in_=bvr[:, :])

            with tc.tile_pool(name="kv", bufs=1) as kvp:
                kt = [kvp.tile([128, kmax], F32R, tag=f"kt{m}", name=f"kt{m}")
                      for m in range(NCH)]
                vt = [kvp.tile([128, N_HEADS, D_HEAD + 1], BF16, tag=f"vt{kb}", name=f"vt{kb}")
                      for kb in range(nkb_all)]

                # ============ projection phases ============
                with (
                    tc.tile_pool(name="w", bufs=1) as wp,
                    tc.tile_pool(name="xts", bufs=2) as xp,
                    tc.tile_pool(name="pj", bufs=4, space="PSUM") as pj,
                ):
                    def load_w(name):
                        ts = []
                        for c in range(NCH):
                            t = wp.tile([128, D_MODEL], F32R, tag=f"w{c}", name=f"w_{c}")
                            nc.sync.dma_start(out=t, in_=w_c[name][c])
                            ts.append(t)
                        return ts

                    def load_x(kc0, kcw):
                        ts = []
                        for c in range(NCH):
                            t = xp.tile([128, kcw], F32R, tag=f"x{c}", name=f"x_{c}")
                            nc.sync.dma_start(out=t, in_=xt_c[c][:, kc0:kc0 + kcw])
                            ts.append(t)
                        return ts

                    # K projection -> SBUF kt tiles
                    w_tiles = load_w("k")
                    for kc0, kcw in kcs:
                        x_tiles = load_x(kc0, kcw)
                        for m in range(NCH):
                            ps = pj.tile([128, kcw], F32, tag="pk")
                            for c in range(NCH):
                                nc.tensor.matmul(
                                    ps, w_tiles[c][:, m * 128:(m + 1) * 128],
                                    x_tiles[c],
                                    start=(c == 0), stop=(c == NCH - 1))
                            nc.scalar.activation(
                                kt[m][:, kc0:kc0 + kcw], ps, Act.Identity,
                                bias=bk_t[:, m:m + 1])

                    # Q projection -> DRAM spill
                    w_tiles = load_w("q")
                    for qc0, qcw in qbs:
                        x_tiles = load_x(qc0, qcw)
                        for m in range(NCH):
                            ps = pj.tile([128, qcw], F32, tag="pk")
                            for c in range(NCH):
                                nc.tensor.matmul(
                                    ps, w_tiles[c][:, m * 128:(m + 1) * 128],
                                    x_tiles[c],
                                    start=(c == 0), stop=(c == NCH - 1))
                            qs_t = stg.tile([128, qcw], F32R, tag="spill")
                            nc.scalar.activation(
                                qs_t, ps, Act.Identity, bias=bq_t[:, m:m + 1])
                            nc.sync.dma_start(
                                out=qdram[m * 128:(m + 1) * 128,
                                          qc0 - q0:qc0 - q0 + qcw],
                                in_=qs_t)

                    # V projection -> SBUF vt tiles (bf16, ones col at 64)
                    w_tiles = load_w("v")
                    for kc0, kcw in kcs:
                        x_tiles = load_x(kc0, kcw)
                        for kbl in range(kcw // 128):
                            kb = kc0 // 128 + kbl
                            nc.vector.memset(
                                vt[kb][:, :, D_HEAD:D_HEAD + 1], 1.0)
                            for dc in range(2):
                                ps = pj.tile([128, 512], F32, tag="pk")
                                for c in range(NCH):
                                    nc.tensor.matmul(
                                        ps,
                                        x_tiles[c][:, kbl * 128:(kbl + 1) * 128],
                                        w_tiles[c][:, dc * 512:(dc + 1) * 512],
                                        start=(c == 0), stop=(c == NCH - 1))
                                nc.vector.scalar_tensor_tensor(
                                    out=vt[kb][:, dc * 8:(dc + 1) * 8, 0:D_HEAD],
                                    in0=ps.rearrange("p (h d) -> p h d", d=D_HEAD),
                                    scalar=0.0,
                                    in1=bvr_t[:, dc * 512:(dc + 1) * 512]
                                        .rearrange("p (h d) -> p h d", d=D_HEAD),
                                    op0=Alu.add, op1=Alu.add)

                # ============ attention ============
                scale = 1.0 / math.sqrt(D_HEAD)
                with (
                    tc.tile_pool(name="qtq", bufs=2) as qtp,
                    tc.tile_pool(name="pt", bufs=1) as ptp,
                    tc.tile_pool(name="ytq", bufs=2) as ytp,
                    tc.tile_pool(name="ps_s", bufs=3, space="PSUM") as pss,
                    tc.tile_pool(name="ps_o", bufs=2, space="PSUM") as pso,
                ):
                    for qs, qw in qbs:
                        qoff = qs - q0
                        nkb = (qs + qw) // 128
                        qtq = []
                        for m in range(NCH):
                            t = qtp.tile([128, qw], F32R, tag=f"q{m}", name=f"qtq{m}")
                            nc.sync.dma_start(
                                out=t,
                                in_=qdram[m * 128:(m + 1) * 128, qoff:qoff + qw])
                            qtq.append(t)
                        ytq = [ytp.tile([128, qw], F32R, tag=f"y{m}", name=f"ytq{m}")
                               for m in range(NCH)]
                        for h in range(N_HEADS):
                            hm, hr = h // 2, (h % 2) * 64
                            pts = []
                            for kb in range(nkb):
                                c0 = max(0, kb * 128 - qs)
                                sp = pss.tile([128, qw - c0], F32, tag="s")
                                nc.tensor.matmul(
                                    sp,
                                    kt[hm][hr:hr + 64, kb * 128:(kb + 1) * 128],
                                    qtq[hm][hr:hr + 64, c0:qw],
                                    start=True, stop=True)
                                pt = ptp.tile([128, qw], BF16, tag=f"pt{kb}", name=f"pt_{kb}")
                                pts.append(pt)
                                nc.scalar.activation(
                                    pt[:, c0:qw], sp, Act.Exp,
                                    bias=kbias_t[:, kb:kb + 1], scale=scale)
                                if kb * 128 >= qs:
                                    dw = min(128, qw - c0)
                                    nc.vector.tensor_tensor(
                                        out=pt[:, c0:c0 + dw],
                                        in0=pt[:, c0:c0 + dw],
                                        in1=tri_t[:, 0:dw], op=Alu.mult)
                            op = pso.tile([65, qw], F32, tag="o")
                            for kb in range(nkb):
                                c0 = max(0, kb * 128 - qs)
                                nc.tensor.matmul(
                                    op[:, c0:qw], vt[kb][:, h, :],
                                    pts[kb][:, c0:qw],
                                    start=(kb == 0), stop=(kb == nkb - 1))
                            osb = stg.tile([65, qw], F32, tag="osb")
                            nc.scalar.copy(osb, op)
                            rc = stg.tile([1, qw], F32, tag="rc")
                            nc.vector.reciprocal(rc, osb[64:65, :])
                            dr = stg.tile([64, qw], F32, tag="dr")
                            nc.gpsimd.partition_broadcast(dr, rc)
                            nc.vector.tensor_tensor(
                                out=ytq[hm][hr:hr + 64, :],
                                in0=osb[0:64, :], in1=dr, op=Alu.mult)
                        for m in range(NCH):
                            nc.sync.dma_start(
                                out=ydram[m * 128:(m + 1) * 128, qoff:qoff + qw],
                                in_=ytq[m])

            # ============ output projection ============
            with (
                tc.tile_pool(name="w2", bufs=1) as wp2,
                tc.tile_pool(name="yin", bufs=2) as yip,
                tc.tile_pool(name="po", bufs=4, space="PSUM") as po,
            ):
                bor_t = wp2.tile([128, D_MODEL], F32, tag="bor")
                nc.sync.dma_start(out=bor_t, in_=bor[:, :])
                wo_tiles = []
                for c in range(NCH):
                    t = wp2.tile([128, D_MODEL], F32R, tag=f"w{c}", name=f"wo_{c}")
                    nc.sync.dma_start(out=t, in_=w_c["o"][c])
                    wo_tiles.append(t)
                for ob in range(nq // 128):
                    yin = []
                    for c in range(NCH):
                        t = yip.tile([128, 128], F32R, tag=f"yi{c}", name=f"yi_{c}")
                        nc.sync.dma_start(
                            out=t,
                            in_=ydram[c * 128:(c + 1) * 128,
                                      ob * 128:(ob + 1) * 128])
                        yin.append(t)
                    for oc in range(2):
                        pp = po.tile([128, 512], F32, tag="pp")
                        for c in range(NCH):
                            nc.tensor.matmul(
                                pp, yin[c],
                                wo_tiles[c][:, oc * 512:(oc + 1) * 512],
                                start=(c == 0), stop=(c == NCH - 1))
                        ost = stg.tile([128, 512], F32, tag="ost")
                        nc.vector.scalar_tensor_tensor(
                            out=ost, in0=pp, scalar=0.0,
                            in1=bor_t[:, oc * 512:(oc + 1) * 512],
                            op0=Alu.add, op1=Alu.add)
                        nc.sync.dma_start(
                            out=out[ob * 128:(ob + 1) * 128,
                                    oc * 512:(oc + 1) * 512],
                            in_=ost)
    nc.compile()
    return nc


# ---------------------------------------------------------------- run path

def _make_sharded(nc, devices):
    """jitted shard_map over an explicit device subset (adapted from
    bass2jax.run_bass_via_pjrt)."""
    bass2jax.install_neuronx_cc_hook()
    partition_name = nc.partition_id_tensor.name if nc.partition_id_tensor else None

    in_names, out_names, out_avals, zero_shapes = [], [], [], []
    for alloc in nc.m.functions[0].allocations:
        if not isinstance(alloc, mybir.MemoryLocationSet):
            continue
        name = alloc.memorylocations[0].name
        if alloc.kind == "ExternalInput":
            if name != partition_name:
                in_names.append(name)
        elif alloc.kind == "ExternalOutput":
            out_names.append(name)
            shape = tuple(alloc.tensor_shape)
            dtype = mybir.dt.np(alloc.dtype)
            out_avals.append(jax.core.ShapedArray(shape, dtype))
            zero_shapes.append((shape, dtype))
    n_params = len(in_names)
    all_in_names = list(in_names) + list(out_names)
    if partition_name is not None:
        all_in_names.append(partition_name)

    def _body(*args):
        operands = list(args)
        if partition_name is not None:
            operands.append(bass2jax.partition_id_tensor())
        return tuple(bass2jax._bass_exec_p.bind(
            *operands,
            out_avals=tuple(out_avals),
            in_names=tuple(all_in_names),
            out_names=tuple(out_names),
            lowering_input_output_aliases=(),
            sim_require_finite=True,
            sim_require_nnan=True,
            nc=nc,
        ))

    mesh = Mesh(np.asarray(devices), ("core",))
    n_outs = len(out_names)
    smap_kwargs = dict(
        mesh=mesh,
        in_specs=(PartitionSpec("core"),) * (n_params + n_outs),
        out_specs=(PartitionSpec("core"),) * n_outs)
    try:
        smapped = shard_map(_body, check_vma=False, **smap_kwargs)
    except TypeError:
        smapped = shard_map(_body, check_rep=False, **smap_kwargs)
    fn = jax.jit(
        smapped,
        donate_argnums=tuple(range(n_params, n_params + n_outs)),
        keep_unused=True)

    def run_async(in_maps):
        ncore = len(in_maps)
        concat_in = [np.concatenate([np.asarray(m[n]) for m in in_maps])
                     for n in in_names]
        concat_zeros = [np.zeros((ncore * s[0], *s[1:]), d)
                        for s, d in zero_shapes]
        return fn(*concat_in, *concat_zeros)

    def collect(out_arrs, ncore):
        return [
            {name: np.asarray(out_arrs[i]).reshape(ncore, *out_avals[i].shape)[c]
             for i, name in enumerate(out_names)}
            for c in range(ncore)
        ]

    return run_async, collect


_lock = threading.Lock()
_cached = {}


def _get_runners():
    with _lock:
        if "run" not in _cached:
            devs = jax.devices()
            ncA = build_role(0, CUT, CUT)
            ncB = build_role(CUT, L, L)
            runA, colA = _make_sharded(ncA, devs[0:4])
            runB, colB = _make_sharded(ncB, devs[4:8])
            _cached["run"] = (runA, colA, runB, colB)
    return _cached["run"]


def _host_inputs(x, attn_mask, Wq, bq, Wk, bk, Wv, bv, Wo, bo):
    f32 = np.float32
    shared = dict(
        wkt=np.ascontiguousarray(np.asarray(Wk, f32).T),
        wqt=np.ascontiguousarray(np.asarray(Wq, f32).T),
        wvt=np.ascontiguousarray(np.asarray(Wv, f32).T),
        wot=np.ascontiguousarray(np.asarray(Wo, f32).T),
        bk2=np.ascontiguousarray(np.asarray(bk, f32).reshape(NCH, 128).T),
        bq2=np.ascontiguousarray(np.asarray(bq, f32).reshape(NCH, 128).T),
        bvr=np.ascontiguousarray(
            np.broadcast_to(np.asarray(bv, f32)[None, :], (128, D_MODEL))),
        bor=np.ascontiguousarray(
            np.broadcast_to(np.asarray(bo, f32)[None, :], (128, D_MODEL))),
        tri=(np.arange(128)[:, None] <= np.arange(128)[None, :])
            .astype(ml_dtypes.bfloat16),
    )
    xts = [np.ascontiguousarray(np.asarray(x[b], f32).T) for b in range(B)]
    kb_host = np.where(np.asarray(attn_mask) != 0, 0.0, -1e30).astype(f32)
    kbias = [np.ascontiguousarray(kb_host[b].reshape(16, 128).T)
             for b in range(B)]
    return shared, xts, kbias


def kernel(x, attn_mask, Wq, bq, Wk, bk, Wv, bv, Wo, bo):
    runA, colA, runB, colB = _get_runners()
    shared, xts, kbias = _host_inputs(
        x, attn_mask, Wq, bq, Wk, bk, Wv, bv, Wo, bo)

    mapsA = [dict(shared, xt=np.ascontiguousarray(xts[b][:, :CUT]),
                  kbias=kbias[b]) for b in range(B)]
    mapsB = [dict(shared, xt=xts[b], kbias=kbias[b]) for b in range(B)]

    outA = runA(mapsA)
    outB = runB(mapsB)
    jax.block_until_ready(outA)
    jax.block_until_ready(outB)
    resA = colA(outA, B)
    resB = colB(outB, B)

    out = np.empty((B, L, D_MODEL), np.float32)
    for b in range(B):
        out[b, :CUT] = resA[b]["out"]
        out[b, CUT:] = resB[b]["out"]
    return out


# revision 21
# speedup vs baseline: 1.1507x; 1.1507x over previous
"""Multi-head self-attention (B=4, L=2048, D=1024, H=16, causal + key padding
mask) on 8 Trainium2 NeuronCores.

Sharding: data-parallel over the 4 batches x 2-way split of the query
sequence per batch. The query split is asymmetric ([0:1408) / [1408:2048))
so the causal-attention FLOPs balance: the early-query core needs K/V only
for keys [0:1408) while the late-query core computes K/V for the full
sequence. No cross-core communication; outputs are disjoint row ranges and
the host concatenates them.

Per-core kernel (PE matmuls in fp32r except attention-weights @ V in bf16):
  K^T[d, key] = Wk^T.T @ x^T (+bk per partition), kept in SBUF.
  Q^T likewise, spilled to DRAM and re-read per query block.
  V~[key, h, 0:65] = x^T.T @ Wv^T (+bv) with a ones column per head, so the
  attention @ V matmul also emits the softmax denominator for free.
  Scores are computed transposed, S^T[k, q] = K^T.T @ Q^T, so softmax
  normalization is per-partition and P^T is born in the layout the AV
  matmul wants (zero transposes in the hot path). exp on ScalarE with
  scale=1/8 and the key-padding mask folded into the per-partition bias;
  causal mask = a 0/1 lower-tri multiply on the one diagonal 128-block;
  fully-masked key blocks are skipped outright.
  O~^T[0:65, q] accumulates V~.T @ P^T; row 64 is the denominator.
  Normalize via reciprocal + gpsimd partition-broadcast + multiply into
  y^T (exactly the stationary layout the output projection needs),
  spill y^T to DRAM, then out = y^T.T @ Wo^T + bo.
"""
import math
import threading

import numpy as np
import ml_dtypes

import jax
from jax.sharding import Mesh, PartitionSpec

try:  # jax>=0.8 moved shard_map
    from jax import shard_map
except ImportError:
    from jax.experimental.shard_map import shard_map

import concourse.bass as bass
import concourse.mybir as mybir
import concourse.tile as tile
from concourse import bacc, bass2jax

F32 = mybir.dt.float32
F32R = mybir.dt.float32r
BF16 = mybir.dt.bfloat16
Act = mybir.ActivationFunctionType
Alu = mybir.AluOpType

D_MODEL = 1024
N_HEADS = 16
D_HEAD = 64
B = 4
L = 2048
CUT = 1408  # query split point: balances causal FLOPs across the 2 roles
NCH = D_MODEL // 128


def _r(ap):
    return ap.bitcast(F32R)


def _chunks(lo, hi, step=None):
    """Split [lo, hi) into chunks of width in [256, 512] (>=256 keeps the
    fp32r fast path, <=512 fits one PSUM bank). Prefers 512-wide chunks;
    falls back to 384 when the 512 tail would be <256."""
    if step is None:
        tail = (hi - lo) % 512
        step = 512 if (tail == 0 or tail >= 256) else 384
    out = []
    while lo < hi:
        w = min(step, hi - lo)
        out.append((lo, w))
        lo += w
    assert all(256 <= w <= 512 for _, w in out), out
    return out


def build_role(q0, q1, kmax):
    """Emit the bass program for one role (query rows [q0:q1), keys [0:kmax))."""
    nq = q1 - q0
    nkb_all = kmax // 128
    qbs = _chunks(q0, q1)
    kcs = _chunks(0, kmax)

    nc = bacc.Bacc(None)
    xt = nc.dram_tensor("xt", [D_MODEL, kmax], F32R, kind="ExternalInput")
    wkt = nc.dram_tensor("wkt", [D_MODEL, D_MODEL], F32R, kind="ExternalInput")
    wqt = nc.dram_tensor("wqt", [D_MODEL, D_MODEL], F32R, kind="ExternalInput")
    wvt = nc.dram_tensor("wvt", [D_MODEL, D_MODEL], F32R, kind="ExternalInput")
    wot = nc.dram_tensor("wot", [D_MODEL, D_MODEL], F32R, kind="ExternalInput")
    bk2 = nc.dram_tensor("bk2", [128, NCH], F32, kind="ExternalInput")
    bq2 = nc.dram_tensor("bq2", [128, NCH], F32, kind="ExternalInput")
    bvr = nc.dram_tensor("bvr", [128, D_MODEL], F32, kind="ExternalInput")
    bor = nc.dram_tensor("bor", [128, D_MODEL], F32, kind="ExternalInput")
    tri = nc.dram_tensor("tri", [128, 128], BF16, kind="ExternalInput")
    kbias = nc.dram_tensor("kbias", [128, 16], F32, kind="ExternalInput")
    out = nc.dram_tensor("out", [nq, D_MODEL], F32, kind="ExternalOutput")
    qdram = nc.dram_tensor("qdram", [D_MODEL, nq], F32R)
    ydram = nc.dram_tensor("ydram", [D_MODEL, nq], F32R)
    recdram = nc.dram_tensor("recdram", [16, 512], F32)

    xt_c = xt.rearrange("(c p) k -> c p k", p=128)
    w_c = {n: t.rearrange("(c p) m -> c p m", p=128)
           for n, t in [("k", wkt), ("q", wqt), ("v", wvt), ("o", wot)]}

    with tile.TileContext(nc) as tc:
        with (
            tc.tile_pool(name="consts", bufs=1) as cst,
            tc.tile_pool(name="stage", bufs=3) as stg,
        ):
            # ---- constants ----
            tri_t = cst.tile([128, 128], BF16, tag="tri")
            nc.sync.dma_start(out=tri_t, in_=tri[:, :])
            kbias_t = cst.tile([128, 16], F32, tag="kbias")
            nc.sync.dma_start(out=kbias_t, in_=kbias[:, :])
            bk_t = cst.tile([128, NCH], F32, tag="bk")
            nc.sync.dma_start(out=bk_t, in_=bk2[:, :])
            bq_t = cst.tile([128, NCH], F32, tag="bq")
            nc.sync.dma_start(out=bq_t, in_=bq2[:, :])
            bvr_t = cst.tile([128, D_MODEL], F32, tag="bvr")
            nc.sync.dma_start(out=bvr_t, in_=bvr[:, :])

            with tc.tile_pool(name="kv", bufs=1) as kvp:
                kt = [kvp.tile([128, kmax], F32R, tag=f"kt{m}", name=f"kt{m}")
                      for m in range(NCH)]
                vt = [kvp.tile([128, N_HEADS, D_HEAD + 1], BF16, tag=f"vt{kb}", name=f"vt{kb}")
                      for kb in range(nkb_all)]

                # ============ projection phases ============
                with (
                    tc.tile_pool(name="w", bufs=1) as wp,
                    tc.tile_pool(name="xts", bufs=2) as xp,
                    tc.tile_pool(name="pj", bufs=6, space="PSUM") as pj,
                ):
                    def load_w(name):
                        ts = []
                        for c in range(NCH):
                            t = wp.tile([128, D_MODEL], F32R, tag=f"w{c}", name=f"w_{c}")
                            nc.sync.dma_start(out=t, in_=w_c[name][c])
                            ts.append(t)
                        return ts

                    def load_x(kc0, kcw):
                        ts = []
                        for c in range(NCH):
                            t = xp.tile([128, kcw], F32R, tag=f"x{c}", name=f"x_{c}")
                            nc.sync.dma_start(out=t, in_=xt_c[c][:, kc0:kc0 + kcw])
                            ts.append(t)
                        return ts

                    # K projection -> SBUF kt tiles
                    w_tiles = load_w("k")
                    for kc0, kcw in kcs:
                        x_tiles = load_x(kc0, kcw)
                        for m in range(NCH):
                            ps = pj.tile([128, kcw], F32, tag="pk")
                            for c in range(NCH):
                                nc.tensor.matmul(
                                    ps, w_tiles[c][:, m * 128:(m + 1) * 128],
                                    x_tiles[c],
                                    start=(c == 0), stop=(c == NCH - 1))
                            nc.vector.tensor_scalar(
                                out=kt[m][:, kc0:kc0 + kcw], in0=ps,
                                scalar1=bk_t[:, m:m + 1], scalar2=None,
                                op0=Alu.add)

                    # Q projection -> DRAM spill
                    w_tiles = load_w("q")
                    for qc0, qcw in qbs:
                        x_tiles = load_x(qc0, qcw)
                        for m in range(NCH):
                            ps = pj.tile([128, qcw], F32, tag="pk")
                            for c in range(NCH):
                                nc.tensor.matmul(
                                    ps, w_tiles[c][:, m * 128:(m + 1) * 128],
                                    x_tiles[c],
                                    start=(c == 0), stop=(c == NCH - 1))
                            qs_t = stg.tile([128, qcw], F32R, tag="spill")
                            nc.vector.tensor_scalar(
                                out=qs_t, in0=ps,
                                scalar1=bq_t[:, m:m + 1], scalar2=None,
                                op0=Alu.add)
                            nc.sync.dma_start(
                                out=qdram[m * 128:(m + 1) * 128,
                                          qc0 - q0:qc0 - q0 + qcw],
                                in_=qs_t)

                    # V projection -> SBUF vt tiles (bf16, ones col at 64)
                    w_tiles = load_w("v")
                    for kc0, kcw in kcs:
                        x_tiles = load_x(kc0, kcw)
                        for kbl in range(kcw // 128):
                            kb = kc0 // 128 + kbl
                            nc.vector.memset(
                                vt[kb][:, :, D_HEAD:D_HEAD + 1], 1.0)
                            for dc in range(2):
                                ps = pj.tile([128, 512], F32, tag="pk")
                                for c in range(NCH):
                                    nc.tensor.matmul(
                                        ps,
                                        x_tiles[c][:, kbl * 128:(kbl + 1) * 128],
                                        w_tiles[c][:, dc * 512:(dc + 1) * 512],
                                        start=(c == 0), stop=(c == NCH - 1))
                                nc.vector.scalar_tensor_tensor(
                                    out=vt[kb][:, dc * 8:(dc + 1) * 8, 0:D_HEAD],
                                    in0=ps.rearrange("p (h d) -> p h d", d=D_HEAD),
                                    scalar=0.0,
                                    in1=bvr_t[:, dc * 512:(dc + 1) * 512]
                                        .rearrange("p (h d) -> p h d", d=D_HEAD),
                                    op0=Alu.add, op1=Alu.add)
                            # key-padding: zero masked keys' V rows (and the
                            # ones column, so the denominator drops them too)
                            nc.vector.tensor_scalar(
                                out=vt[kb][:, :, :], in0=vt[kb][:, :, :],
                                scalar1=kbias_t[:, kb:kb + 1], scalar2=None,
                                op0=Alu.mult)

                # ============ attention ============
                scale = 1.0 / math.sqrt(D_HEAD)
                with (
                    tc.tile_pool(name="qtq", bufs=2) as qtp,
                    tc.tile_pool(name="pt", bufs=2) as ptp,
                    tc.tile_pool(name="ytq", bufs=2) as ytp,
                    tc.tile_pool(name="ps_s", bufs=3, space="PSUM") as pss,
                    tc.tile_pool(name="ps_o", bufs=2, space="PSUM") as pso,
                ):
                    for qs, qw in qbs:
                        qoff = qs - q0
                        nkb = (qs + qw) // 128
                        qtq = []
                        for m in range(NCH):
                            t = qtp.tile([128, qw], F32R, tag=f"q{m}", name=f"qtq{m}")
                            nc.sync.dma_start(
                                out=t,
                                in_=qdram[m * 128:(m + 1) * 128, qoff:qoff + qw])
                            qtq.append(t)
                        ytq = [ytp.tile([128, qw], F32R, tag=f"y{m}", name=f"ytq{m}")
                               for m in range(NCH)]
                        # all 16 heads' denominator rows collect into a
                        # [128, 16*qw/128] tile so ONE reciprocal (the DVE's
                        # slowest op, ~6cyc/elem on the free dim) covers them
                        xw = 16 * qw // 128
                        den128 = ytp.tile([128, xw], F32, tag="den",
                                          name="den128", bufs=1)
                        # group key blocks in pairs where both are fully
                        # below the diagonal (c0 == 0) -> one 2-bank PSUM tile
                        # and ONE exp per pair (halves ScalarE op overhead)
                        pairable = [kb for kb in range(nkb) if kb * 128 <= qs]
                        rest = [kb for kb in range(nkb) if kb * 128 > qs]
                        groups = [pairable[i:i + 2]
                                  for i in range(0, len(pairable), 2)]
                        groups += [[kb] for kb in rest]
                        for h in range(N_HEADS):
                            hm, hr = h // 2, (h % 2) * 64
                            pt_of = {}
                            for gi, g in enumerate(groups):
                                sp = pss.tile([128, 2, 512], F32, tag="s")
                                ptg = ptp.tile([128, 2, qw], BF16,
                                               tag=f"pt{gi}", name=f"pt_{gi}")
                                for j, kb in enumerate(g):
                                    c0 = max(0, kb * 128 - qs)
                                    pt_of[kb] = (ptg, j, c0)
                                    nc.tensor.matmul(
                                        sp[:, j, c0:qw],
                                        kt[hm][hr:hr + 64,
                                               kb * 128:(kb + 1) * 128],
                                        qtq[hm][hr:hr + 64, c0:qw],
                                        start=True, stop=True)
                                if len(g) == 2:
                                    nc.scalar.activation(
                                        ptg[:, :, :], sp[:, :, 0:qw], Act.Exp,
                                        scale=scale)
                                else:
                                    c0 = max(0, g[0] * 128 - qs)
                                    nc.scalar.activation(
                                        ptg[:, 0, c0:qw], sp[:, 0, c0:qw],
                                        Act.Exp, scale=scale)
                                for j, kb in enumerate(g):
                                    if kb * 128 >= qs:
                                        c0 = max(0, kb * 128 - qs)
                                        dw = min(128, qw - c0)
                                        nc.vector.tensor_tensor(
                                            out=ptg[:, j, c0:c0 + dw],
                                            in0=ptg[:, j, c0:c0 + dw],
                                            in1=tri_t[:, 0:dw], op=Alu.mult)
                            op = pso.tile([65, qw], F32, tag="o")
                            for kb in range(nkb):
                                ptg, j, c0 = pt_of[kb]
                                nc.tensor.matmul(
                                    op[:, c0:qw], vt[kb][:, h, :],
                                    ptg[:, j, c0:qw],
                                    start=(kb == 0), stop=(kb == nkb - 1))
                            nc.vector.tensor_copy(
                                out=ytq[hm][hr:hr + 64, :], in_=op[0:64, :])
                            rc = stg.tile([1, qw], F32, tag="rc")
                            nc.vector.tensor_copy(out=rc, in_=op[64:65, :])
                            nc.sync.dma_start(
                                out=den128[h * 8:(h + 1) * 8, :], in_=rc)
                        nc.vector.reciprocal(den128, den128)
                        # DRAM round trip so the reload can replicate each
                        # head's row across 64 partitions (DMA broadcast APs
                        # need a DRAM source)
                        for h in range(N_HEADS):
                            nc.sync.dma_start(
                                out=recdram[h, 0:qw].rearrange(
                                    "(b w) -> b w", w=xw),
                                in_=den128[h * 8:(h + 1) * 8, :])
                        for m in range(NCH):
                            rr = stg.tile([128, qw], F32, tag="rr")
                            for half in range(2):
                                h = 2 * m + half
                                src = recdram[h:h + 1, 0:qw]
                                bc = bass.AP(
                                    tensor=src.tensor, offset=src.offset,
                                    ap=[[0, 64]] + [list(d) for d in src.ap[1:]])
                                nc.sync.dma_start(
                                    out=rr[64 * half:64 * half + 64, :], in_=bc)
                            nc.vector.tensor_tensor(
                                out=ytq[m], in0=ytq[m], in1=rr, op=Alu.mult)
                        for m in range(NCH):
                            nc.sync.dma_start(
                                out=ydram[m * 128:(m + 1) * 128, qoff:qoff + qw],
                                in_=ytq[m])

            # ============ output projection ============
            with (
                tc.tile_pool(name="w2", bufs=1) as wp2,
                tc.tile_pool(name="yin", bufs=2) as yip,
                tc.tile_pool(name="po", bufs=4, space="PSUM") as po,
            ):
                bor_t = wp2.tile([128, D_MODEL], F32, tag="bor")
                nc.sync.dma_start(out=bor_t, in_=bor[:, :])
                wo_tiles = []
                for c in range(NCH):
                    t = wp2.tile([128, D_MODEL], F32R, tag=f"w{c}", name=f"wo_{c}")
                    nc.sync.dma_start(out=t, in_=w_c["o"][c])
                    wo_tiles.append(t)
                for ob in range(nq // 128):
                    yin = []
                    for c in range(NCH):
                        t = yip.tile([128, 128], F32R, tag=f"yi{c}", name=f"yi_{c}")
                        nc.sync.dma_start(
                            out=t,
                            in_=ydram[c * 128:(c + 1) * 128,
                                      ob * 128:(ob + 1) * 128])
                        yin.append(t)
                    for oc in range(2):
                        pp = po.tile([128, 512], F32, tag="pp")
                        for c in range(NCH):
                            nc.tensor.matmul(
                                pp, yin[c],
                                wo_tiles[c][:, oc * 512:(oc + 1) * 512],
                                start=(c == 0), stop=(c == NCH - 1))
                        ost = stg.tile([128, 512], F32, tag="ost")
                        nc.vector.scalar_tensor_tensor(
                            out=ost, in0=pp, scalar=0.0,
                            in1=bor_t[:, oc * 512:(oc + 1) * 512],
                            op0=Alu.add, op1=Alu.add)
                        nc.sync.dma_start(
                            out=out[ob * 128:(ob + 1) * 128,
                                    oc * 512:(oc + 1) * 512],
                            in_=ost)
    nc.compile()
    return nc


# ---------------------------------------------------------------- run path

def _make_sharded(nc, devices):
    """jitted shard_map over an explicit device subset (adapted from
    bass2jax.run_bass_via_pjrt)."""
    bass2jax.install_neuronx_cc_hook()
    partition_name = nc.partition_id_tensor.name if nc.partition_id_tensor else None

    in_names, out_names, out_avals, zero_shapes = [], [], [], []
    for alloc in nc.m.functions[0].allocations:
        if not isinstance(alloc, mybir.MemoryLocationSet):
            continue
        name = alloc.memorylocations[0].name
        if alloc.kind == "ExternalInput":
            if name != partition_name:
                in_names.append(name)
        elif alloc.kind == "ExternalOutput":
            out_names.append(name)
            shape = tuple(alloc.tensor_shape)
            dtype = mybir.dt.np(alloc.dtype)
            out_avals.append(jax.core.ShapedArray(shape, dtype))
            zero_shapes.append((shape, dtype))
    n_params = len(in_names)
    all_in_names = list(in_names) + list(out_names)
    if partition_name is not None:
        all_in_names.append(partition_name)

    def _body(*args):
        operands = list(args)
        if partition_name is not None:
            operands.append(bass2jax.partition_id_tensor())
        return tuple(bass2jax._bass_exec_p.bind(
            *operands,
            out_avals=tuple(out_avals),
            in_names=tuple(all_in_names),
            out_names=tuple(out_names),
            lowering_input_output_aliases=(),
            sim_require_finite=True,
            sim_require_nnan=True,
            nc=nc,
        ))

    mesh = Mesh(np.asarray(devices), ("core",))
    n_outs = len(out_names)
    smap_kwargs = dict(
        mesh=mesh,
        in_specs=(PartitionSpec("core"),) * (n_params + n_outs),
        out_specs=(PartitionSpec("core"),) * n_outs)
    try:
        smapped = shard_map(_body, check_vma=False, **smap_kwargs)
    except TypeError:
        smapped = shard_map(_body, check_rep=False, **smap_kwargs)
    fn = jax.jit(
        smapped,
        donate_argnums=tuple(range(n_params, n_params + n_outs)),
        keep_unused=True)

    def run_async(in_maps):
        ncore = len(in_maps)
        concat_in = [np.concatenate([np.asarray(m[n]) for m in in_maps])
                     for n in in_names]
        concat_zeros = [np.zeros((ncore * s[0], *s[1:]), d)
                        for s, d in zero_shapes]
        return fn(*concat_in, *concat_zeros)

    def collect(out_arrs, ncore):
        return [
            {name: np.asarray(out_arrs[i]).reshape(ncore, *out_avals[i].shape)[c]
             for i, name in enumerate(out_names)}
            for c in range(ncore)
        ]

    return run_async, collect


_lock = threading.Lock()
_cached = {}


def _get_runners():
    with _lock:
        if "run" not in _cached:
            devs = jax.devices()
            ncA = build_role(0, CUT, CUT)
            ncB = build_role(CUT, L, L)
            runA, colA = _make_sharded(ncA, devs[0:4])
            runB, colB = _make_sharded(ncB, devs[4:8])
            _cached["run"] = (runA, colA, runB, colB)
    return _cached["run"]


def _host_inputs(x, attn_mask, Wq, bq, Wk, bk, Wv, bv, Wo, bo):
    f32 = np.float32
    shared = dict(
        wkt=np.ascontiguousarray(np.asarray(Wk, f32).T),
        wqt=np.ascontiguousarray(np.asarray(Wq, f32).T),
        wvt=np.ascontiguousarray(np.asarray(Wv, f32).T),
        wot=np.ascontiguousarray(np.asarray(Wo, f32).T),
        bk2=np.ascontiguousarray(np.asarray(bk, f32).reshape(NCH, 128).T),
        bq2=np.ascontiguousarray(np.asarray(bq, f32).reshape(NCH, 128).T),
        bvr=np.ascontiguousarray(
            np.broadcast_to(np.asarray(bv, f32)[None, :], (128, D_MODEL))),
        bor=np.ascontiguousarray(
            np.broadcast_to(np.asarray(bo, f32)[None, :], (128, D_MODEL))),
        tri=(np.arange(128)[:, None] <= np.arange(128)[None, :])
            .astype(ml_dtypes.bfloat16),
    )
    xts = [np.ascontiguousarray(np.asarray(x[b], f32).T) for b in range(B)]
    kb_host = np.where(np.asarray(attn_mask) != 0, 1.0, 0.0).astype(f32)
    kbias = [np.ascontiguousarray(kb_host[b].reshape(16, 128).T)
             for b in range(B)]
    return shared, xts, kbias


def kernel(x, attn_mask, Wq, bq, Wk, bk, Wv, bv, Wo, bo):
    runA, colA, runB, colB = _get_runners()
    shared, xts, kbias = _host_inputs(
        x, attn_mask, Wq, bq, Wk, bk, Wv, bv, Wo, bo)

    mapsA = [dict(shared, xt=np.ascontiguousarray(xts[b][:, :CUT]),
                  kbias=kbias[b]) for b in range(B)]
    mapsB = [dict(shared, xt=xts[b], kbias=kbias[b]) for b in range(B)]

    outA = runA(mapsA)
    outB = runB(mapsB)
    jax.block_until_ready(outA)
    jax.block_until_ready(outB)
    resA = colA(outA, B)
    resB = colB(outB, B)

    out = np.empty((B, L, D_MODEL), np.float32)
    for b in range(B):
        out[b, :CUT] = resA[b]["out"]
        out[b, CUT:] = resB[b]["out"]
    return out


# revision 22
# speedup vs baseline: 1.4866x; 1.2919x over previous
"""Multi-head self-attention (B=4, L=2048, D=1024, H=16, causal + key padding
mask) on 8 Trainium2 NeuronCores.

Sharding: data-parallel over the 4 batches x 2-way split of the query
sequence per batch. The query split is asymmetric ([0:1408) / [1408:2048))
so the causal-attention FLOPs balance: the early-query core needs K/V only
for keys [0:1408) while the late-query core computes K/V for the full
sequence. No cross-core communication; outputs are disjoint row ranges and
the host concatenates them.

Per-core kernel (PE matmuls in fp32r except attention-weights @ V in bf16):
  K^T[d, key] = Wk^T.T @ x^T (+bk per partition), kept in SBUF.
  Q^T likewise, spilled to DRAM and re-read per query block.
  V~[key, h, 0:65] = x^T.T @ Wv^T (+bv) with a ones column per head, so the
  attention @ V matmul also emits the softmax denominator for free.
  Scores are computed transposed, S^T[k, q] = K^T.T @ Q^T, so softmax
  normalization is per-partition and P^T is born in the layout the AV
  matmul wants (zero transposes in the hot path). exp on ScalarE with
  scale=1/8 and the key-padding mask folded into the per-partition bias;
  causal mask = a 0/1 lower-tri multiply on the one diagonal 128-block;
  fully-masked key blocks are skipped outright.
  O~^T[0:65, q] accumulates V~.T @ P^T; row 64 is the denominator.
  Normalize via reciprocal + gpsimd partition-broadcast + multiply into
  y^T (exactly the stationary layout the output projection needs),
  spill y^T to DRAM, then out = y^T.T @ Wo^T + bo.
"""
import math
import threading

import numpy as np
import ml_dtypes

import jax
from jax.sharding import Mesh, PartitionSpec

try:  # jax>=0.8 moved shard_map
    from jax import shard_map
except ImportError:
    from jax.experimental.shard_map import shard_map

import concourse.bass as bass
import concourse.mybir as mybir
import concourse.tile as tile
from concourse import bacc, bass2jax

F32 = mybir.dt.float32
F32R = mybir.dt.float32r
BF16 = mybir.dt.bfloat16
Act = mybir.ActivationFunctionType
Alu = mybir.AluOpType

D_MODEL = 1024
N_HEADS = 16
D_HEAD = 64
B = 4
L = 2048
CUT = 1408  # query split point: balances causal FLOPs across the 2 roles
NCH = D_MODEL // 128


def _r(ap):
    return ap.bitcast(F32R)


def _chunks(lo, hi, step=None):
    """Split [lo, hi) into chunks of width in [256, 512] (>=256 keeps the
    fp32r fast path, <=512 fits one PSUM bank). Prefers 512-wide chunks;
    falls back to 384 when the 512 tail would be <256."""
    if step is None:
        tail = (hi - lo) % 512
        step = 512 if (tail == 0 or tail >= 256) else 384
    out = []
    while lo < hi:
        w = min(step, hi - lo)
        out.append((lo, w))
        lo += w
    assert all(256 <= w <= 512 for _, w in out), out
    return out


def build_role(q0, q1, kmax):
    """Emit the bass program for one role (query rows [q0:q1), keys [0:kmax))."""
    nq = q1 - q0
    nkb_all = kmax // 128
    qbs = _chunks(q0, q1)
    kcs = _chunks(0, kmax)

    nc = bacc.Bacc(None)
    xt = nc.dram_tensor("xt", [D_MODEL, kmax], F32R, kind="ExternalInput")
    wkt = nc.dram_tensor("wkt", [D_MODEL, D_MODEL], F32R, kind="ExternalInput")
    wqt = nc.dram_tensor("wqt", [D_MODEL, D_MODEL], F32R, kind="ExternalInput")
    wvt = nc.dram_tensor("wvt", [D_MODEL, D_MODEL], F32R, kind="ExternalInput")
    wot = nc.dram_tensor("wot", [D_MODEL, D_MODEL], F32R, kind="ExternalInput")
    bk2 = nc.dram_tensor("bk2", [128, NCH], F32, kind="ExternalInput")
    bq2 = nc.dram_tensor("bq2", [128, NCH], F32, kind="ExternalInput")
    bvr = nc.dram_tensor("bvr", [128, D_MODEL], F32, kind="ExternalInput")
    bor = nc.dram_tensor("bor", [128, D_MODEL], F32, kind="ExternalInput")
    tri = nc.dram_tensor("tri", [128, 128], BF16, kind="ExternalInput")
    kbias = nc.dram_tensor("kbias", [128, 16], F32, kind="ExternalInput")
    out = nc.dram_tensor("out", [nq, D_MODEL], F32, kind="ExternalOutput")
    qdram = nc.dram_tensor("qdram", [D_MODEL, nq], BF16)
    ydram = nc.dram_tensor("ydram", [D_MODEL, nq], F32R)
    recdram = nc.dram_tensor("recdram", [16, 512], F32)

    xt_c = xt.rearrange("(c p) k -> c p k", p=128)
    w_c = {n: t.rearrange("(c p) m -> c p m", p=128)
           for n, t in [("k", wkt), ("q", wqt), ("v", wvt), ("o", wot)]}

    with tile.TileContext(nc) as tc:
        with (
            tc.tile_pool(name="consts", bufs=1) as cst,
            tc.tile_pool(name="stage", bufs=3) as stg,
        ):
            # ---- constants ----
            tri_t = cst.tile([128, 128], BF16, tag="tri")
            nc.sync.dma_start(out=tri_t, in_=tri[:, :])
            kbias_t = cst.tile([128, 16], F32, tag="kbias")
            nc.sync.dma_start(out=kbias_t, in_=kbias[:, :])
            bk_t = cst.tile([128, NCH], F32, tag="bk")
            nc.sync.dma_start(out=bk_t, in_=bk2[:, :])
            bq_t = cst.tile([128, NCH], F32, tag="bq")
            nc.sync.dma_start(out=bq_t, in_=bq2[:, :])
            bvr_t = cst.tile([128, D_MODEL], F32, tag="bvr")
            nc.sync.dma_start(out=bvr_t, in_=bvr[:, :])

            with tc.tile_pool(name="kv", bufs=1) as kvp:
                # per-head K^T tiles, bf16: head h occupies its natural 64
                # rows ((h%2)*64), the other 64 rows are ZERO so the score
                # matmul contracts over all 128 partitions (half-row matmuls
                # keep the PE clock gate at 1.2GHz; see HAM notes).
                kt = [kvp.tile([128, kmax], BF16, tag=f"kt{h}", name=f"kt{h}")
                      for h in range(N_HEADS)]
                for h in range(N_HEADS):
                    z0 = 64 if h % 2 == 0 else 0
                    nc.gpsimd.memset(kt[h][z0:z0 + 64, :], 0.0)
                vt = [kvp.tile([128, N_HEADS, D_HEAD + 1], BF16, tag=f"vt{kb}", name=f"vt{kb}")
                      for kb in range(nkb_all)]

                # ============ projection phases ============
                with (
                    tc.tile_pool(name="w", bufs=1) as wp,
                    tc.tile_pool(name="xts", bufs=2) as xp,
                    tc.tile_pool(name="pj", bufs=6, space="PSUM") as pj,
                ):
                    def load_w(name):
                        ts = []
                        for c in range(NCH):
                            t = wp.tile([128, D_MODEL], F32R, tag=f"w{c}", name=f"w_{c}")
                            nc.sync.dma_start(out=t, in_=w_c[name][c])
                            ts.append(t)
                        return ts

                    def load_x(kc0, kcw):
                        ts = []
                        for c in range(NCH):
                            t = xp.tile([128, kcw], F32R, tag=f"x{c}", name=f"x_{c}")
                            nc.sync.dma_start(out=t, in_=xt_c[c][:, kc0:kc0 + kcw])
                            ts.append(t)
                        return ts

                    # K projection -> SBUF kt tiles
                    w_tiles = load_w("k")
                    for kc0, kcw in kcs:
                        x_tiles = load_x(kc0, kcw)
                        for m in range(NCH):
                            ps = pj.tile([128, kcw], F32, tag="pk")
                            for c in range(NCH):
                                nc.tensor.matmul(
                                    ps, w_tiles[c][:, m * 128:(m + 1) * 128],
                                    x_tiles[c],
                                    start=(c == 0), stop=(c == NCH - 1))
                            for par in range(2):
                                hr_ = par * 64
                                nc.vector.tensor_scalar(
                                    out=kt[2 * m + par][hr_:hr_ + 64,
                                                        kc0:kc0 + kcw],
                                    in0=ps[hr_:hr_ + 64, :],
                                    scalar1=bk_t[hr_:hr_ + 64, m:m + 1],
                                    scalar2=None, op0=Alu.add)

                    # Q projection -> DRAM spill
                    w_tiles = load_w("q")
                    for qc0, qcw in qbs:
                        x_tiles = load_x(qc0, qcw)
                        for m in range(NCH):
                            ps = pj.tile([128, qcw], F32, tag="pk")
                            for c in range(NCH):
                                nc.tensor.matmul(
                                    ps, w_tiles[c][:, m * 128:(m + 1) * 128],
                                    x_tiles[c],
                                    start=(c == 0), stop=(c == NCH - 1))
                            qs_t = stg.tile([128, qcw], BF16, tag="spill")
                            nc.vector.tensor_scalar(
                                out=qs_t, in0=ps,
                                scalar1=bq_t[:, m:m + 1], scalar2=None,
                                op0=Alu.add)
                            nc.sync.dma_start(
                                out=qdram[m * 128:(m + 1) * 128,
                                          qc0 - q0:qc0 - q0 + qcw],
                                in_=qs_t)

                    # V projection -> SBUF vt tiles (bf16, ones col at 64)
                    w_tiles = load_w("v")
                    for kc0, kcw in kcs:
                        x_tiles = load_x(kc0, kcw)
                        for kbl in range(kcw // 128):
                            kb = kc0 // 128 + kbl
                            nc.vector.memset(
                                vt[kb][:, :, D_HEAD:D_HEAD + 1], 1.0)
                            for dc in range(2):
                                ps = pj.tile([128, 512], F32, tag="pk")
                                for c in range(NCH):
                                    nc.tensor.matmul(
                                        ps,
                                        x_tiles[c][:, kbl * 128:(kbl + 1) * 128],
                                        w_tiles[c][:, dc * 512:(dc + 1) * 512],
                                        start=(c == 0), stop=(c == NCH - 1))
                                nc.vector.scalar_tensor_tensor(
                                    out=vt[kb][:, dc * 8:(dc + 1) * 8, 0:D_HEAD],
                                    in0=ps.rearrange("p (h d) -> p h d", d=D_HEAD),
                                    scalar=0.0,
                                    in1=bvr_t[:, dc * 512:(dc + 1) * 512]
                                        .rearrange("p (h d) -> p h d", d=D_HEAD),
                                    op0=Alu.add, op1=Alu.add)
                            # key-padding: zero masked keys' V rows (and the
                            # ones column, so the denominator drops them too)
                            nc.vector.tensor_scalar(
                                out=vt[kb][:, :, :], in0=vt[kb][:, :, :],
                                scalar1=kbias_t[:, kb:kb + 1], scalar2=None,
                                op0=Alu.mult)

                # ============ attention ============
                scale = 1.0 / math.sqrt(D_HEAD)
                with (
                    tc.tile_pool(name="qtq", bufs=2) as qtp,
                    tc.tile_pool(name="pt", bufs=2) as ptp,
                    tc.tile_pool(name="ytq", bufs=2) as ytp,
                    tc.tile_pool(name="ps_s", bufs=3, space="PSUM") as pss,
                    tc.tile_pool(name="ps_o", bufs=2, space="PSUM") as pso,
                ):
                    for qs, qw in qbs:
                        qoff = qs - q0
                        nkb = (qs + qw) // 128
                        qtq = []
                        for m in range(NCH):
                            t = qtp.tile([128, qw], BF16, tag=f"q{m}", name=f"qtq{m}")
                            nc.sync.dma_start(
                                out=t,
                                in_=qdram[m * 128:(m + 1) * 128, qoff:qoff + qw])
                            qtq.append(t)
                        ytq = [ytp.tile([128, qw], F32R, tag=f"y{m}", name=f"ytq{m}")
                               for m in range(NCH)]
                        # all 16 heads' denominator rows collect into a
                        # [128, 16*qw/128] tile so ONE reciprocal (the DVE's
                        # slowest op, ~6cyc/elem on the free dim) covers them
                        xw = 16 * qw // 128
                        den128 = ytp.tile([128, xw], F32, tag="den",
                                          name="den128", bufs=1)
                        # group key blocks in pairs where both are fully
                        # below the diagonal (c0 == 0) -> one 2-bank PSUM tile
                        # and ONE exp per pair (halves ScalarE op overhead)
                        pairable = [kb for kb in range(nkb) if kb * 128 <= qs]
                        rest = [kb for kb in range(nkb) if kb * 128 > qs]
                        groups = [pairable[i:i + 2]
                                  for i in range(0, len(pairable), 2)]
                        groups += [[kb] for kb in rest]
                        for h in range(N_HEADS):
                            hm, hr = h // 2, (h % 2) * 64
                            pt_of = {}
                            for gi, g in enumerate(groups):
                                sp = pss.tile([128, 2, 512], F32, tag="s")
                                ptg = ptp.tile([128, 2, qw], BF16,
                                               tag=f"pt{gi}", name=f"pt_{gi}")
                                for j, kb in enumerate(g):
                                    c0 = max(0, kb * 128 - qs)
                                    pt_of[kb] = (ptg, j, c0)
                                    nc.tensor.matmul(
                                        sp[:, j, c0:qw],
                                        kt[h][:, kb * 128:(kb + 1) * 128],
                                        qtq[hm][:, c0:qw],
                                        start=True, stop=True)
                                if len(g) == 2:
                                    nc.scalar.activation(
                                        ptg[:, :, :], sp[:, :, 0:qw], Act.Exp,
                                        scale=scale)
                                else:
                                    c0 = max(0, g[0] * 128 - qs)
                                    nc.scalar.activation(
                                        ptg[:, 0, c0:qw], sp[:, 0, c0:qw],
                                        Act.Exp, scale=scale)
                                for j, kb in enumerate(g):
                                    if kb * 128 >= qs:
                                        c0 = max(0, kb * 128 - qs)
                                        dw = min(128, qw - c0)
                                        nc.vector.tensor_tensor(
                                            out=ptg[:, j, c0:c0 + dw],
                                            in0=ptg[:, j, c0:c0 + dw],
                                            in1=tri_t[:, 0:dw], op=Alu.mult)
                            op = pso.tile([65, qw], F32, tag="o")
                            for kb in range(nkb):
                                ptg, j, c0 = pt_of[kb]
                                nc.tensor.matmul(
                                    op[:, c0:qw], vt[kb][:, h, :],
                                    ptg[:, j, c0:qw],
                                    start=(kb == 0), stop=(kb == nkb - 1))
                            nc.vector.tensor_copy(
                                out=ytq[hm][hr:hr + 64, :], in_=op[0:64, :])
                            rc = stg.tile([1, qw], F32, tag="rc")
                            nc.vector.tensor_copy(out=rc, in_=op[64:65, :])
                            nc.sync.dma_start(
                                out=den128[h * 8:(h + 1) * 8, :], in_=rc)
                        nc.vector.reciprocal(den128, den128)
                        # DRAM round trip so the reload can replicate each
                        # head's row across 64 partitions (DMA broadcast APs
                        # need a DRAM source)
                        for h in range(N_HEADS):
                            nc.sync.dma_start(
                                out=recdram[h, 0:qw].rearrange(
                                    "(b w) -> b w", w=xw),
                                in_=den128[h * 8:(h + 1) * 8, :])
                        for m in range(NCH):
                            rr = stg.tile([128, qw], F32, tag="rr")
                            for half in range(2):
                                h = 2 * m + half
                                src = recdram[h:h + 1, 0:qw]
                                bc = bass.AP(
                                    tensor=src.tensor, offset=src.offset,
                                    ap=[[0, 64]] + [list(d) for d in src.ap[1:]])
                                nc.sync.dma_start(
                                    out=rr[64 * half:64 * half + 64, :], in_=bc)
                            nc.vector.tensor_tensor(
                                out=ytq[m], in0=ytq[m], in1=rr, op=Alu.mult)
                        for m in range(NCH):
                            nc.sync.dma_start(
                                out=ydram[m * 128:(m + 1) * 128, qoff:qoff + qw],
                                in_=ytq[m])

            # ============ output projection ============
            with (
                tc.tile_pool(name="w2", bufs=1) as wp2,
                tc.tile_pool(name="yin", bufs=2) as yip,
                tc.tile_pool(name="po", bufs=4, space="PSUM") as po,
            ):
                bor_t = wp2.tile([128, D_MODEL], F32, tag="bor")
                nc.sync.dma_start(out=bor_t, in_=bor[:, :])
                wo_tiles = []
                for c in range(NCH):
                    t = wp2.tile([128, D_MODEL], F32R, tag=f"w{c}", name=f"wo_{c}")
                    nc.sync.dma_start(out=t, in_=w_c["o"][c])
                    wo_tiles.append(t)
                for ob in range(nq // 128):
                    yin = []
                    for c in range(NCH):
                        t = yip.tile([128, 128], F32R, tag=f"yi{c}", name=f"yi_{c}")
                        nc.sync.dma_start(
                            out=t,
                            in_=ydram[c * 128:(c + 1) * 128,
                                      ob * 128:(ob + 1) * 128])
                        yin.append(t)
                    for oc in range(2):
                        pp = po.tile([128, 512], F32, tag="pp")
                        for c in range(NCH):
                            nc.tensor.matmul(
                                pp, yin[c],
                                wo_tiles[c][:, oc * 512:(oc + 1) * 512],
                                start=(c == 0), stop=(c == NCH - 1))
                        ost = stg.tile([128, 512], F32, tag="ost")
                        nc.vector.scalar_tensor_tensor(
                            out=ost, in0=pp, scalar=0.0,
                            in1=bor_t[:, oc * 512:(oc + 1) * 512],
                            op0=Alu.add, op1=Alu.add)
                        nc.sync.dma_start(
                            out=out[ob * 128:(ob + 1) * 128,
                                    oc * 512:(oc + 1) * 512],
                            in_=ost)
    nc.compile()
    return nc


# ---------------------------------------------------------------- run path

def _make_sharded(nc, devices):
    """jitted shard_map over an explicit device subset (adapted from
    bass2jax.run_bass_via_pjrt)."""
    bass2jax.install_neuronx_cc_hook()
    partition_name = nc.partition_id_tensor.name if nc.partition_id_tensor else None

    in_names, out_names, out_avals, zero_shapes = [], [], [], []
    for alloc in nc.m.functions[0].allocations:
        if not isinstance(alloc, mybir.MemoryLocationSet):
            continue
        name = alloc.memorylocations[0].name
        if alloc.kind == "ExternalInput":
            if name != partition_name:
                in_names.append(name)
        elif alloc.kind == "ExternalOutput":
            out_names.append(name)
            shape = tuple(alloc.tensor_shape)
            dtype = mybir.dt.np(alloc.dtype)
            out_avals.append(jax.core.ShapedArray(shape, dtype))
            zero_shapes.append((shape, dtype))
    n_params = len(in_names)
    all_in_names = list(in_names) + list(out_names)
    if partition_name is not None:
        all_in_names.append(partition_name)

    def _body(*args):
        operands = list(args)
        if partition_name is not None:
            operands.append(bass2jax.partition_id_tensor())
        return tuple(bass2jax._bass_exec_p.bind(
            *operands,
            out_avals=tuple(out_avals),
            in_names=tuple(all_in_names),
            out_names=tuple(out_names),
            lowering_input_output_aliases=(),
            sim_require_finite=True,
            sim_require_nnan=True,
            nc=nc,
        ))

    mesh = Mesh(np.asarray(devices), ("core",))
    n_outs = len(out_names)
    smap_kwargs = dict(
        mesh=mesh,
        in_specs=(PartitionSpec("core"),) * (n_params + n_outs),
        out_specs=(PartitionSpec("core"),) * n_outs)
    try:
        smapped = shard_map(_body, check_vma=False, **smap_kwargs)
    except TypeError:
        smapped = shard_map(_body, check_rep=False, **smap_kwargs)
    fn = jax.jit(
        smapped,
        donate_argnums=tuple(range(n_params, n_params + n_outs)),
        keep_unused=True)

    def run_async(in_maps):
        ncore = len(in_maps)
        concat_in = [np.concatenate([np.asarray(m[n]) for m in in_maps])
                     for n in in_names]
        concat_zeros = [np.zeros((ncore * s[0], *s[1:]), d)
                        for s, d in zero_shapes]
        return fn(*concat_in, *concat_zeros)

    def collect(out_arrs, ncore):
        return [
            {name: np.asarray(out_arrs[i]).reshape(ncore, *out_avals[i].shape)[c]
             for i, name in enumerate(out_names)}
            for c in range(ncore)
        ]

    return run_async, collect


_lock = threading.Lock()
_cached = {}


def _get_runners():
    with _lock:
        if "run" not in _cached:
            devs = jax.devices()
            ncA = build_role(0, CUT, CUT)
            ncB = build_role(CUT, L, L)
            runA, colA = _make_sharded(ncA, devs[0:4])
            runB, colB = _make_sharded(ncB, devs[4:8])
            _cached["run"] = (runA, colA, runB, colB)
    return _cached["run"]


def _host_inputs(x, attn_mask, Wq, bq, Wk, bk, Wv, bv, Wo, bo):
    f32 = np.float32
    shared = dict(
        wkt=np.ascontiguousarray(np.asarray(Wk, f32).T),
        wqt=np.ascontiguousarray(np.asarray(Wq, f32).T),
        wvt=np.ascontiguousarray(np.asarray(Wv, f32).T),
        wot=np.ascontiguousarray(np.asarray(Wo, f32).T),
        bk2=np.ascontiguousarray(np.asarray(bk, f32).reshape(NCH, 128).T),
        bq2=np.ascontiguousarray(np.asarray(bq, f32).reshape(NCH, 128).T),
        bvr=np.ascontiguousarray(
            np.broadcast_to(np.asarray(bv, f32)[None, :], (128, D_MODEL))),
        bor=np.ascontiguousarray(
            np.broadcast_to(np.asarray(bo, f32)[None, :], (128, D_MODEL))),
        tri=(np.arange(128)[:, None] <= np.arange(128)[None, :])
            .astype(ml_dtypes.bfloat16),
    )
    xts = [np.ascontiguousarray(np.asarray(x[b], f32).T) for b in range(B)]
    kb_host = np.where(np.asarray(attn_mask) != 0, 1.0, 0.0).astype(f32)
    kbias = [np.ascontiguousarray(kb_host[b].reshape(16, 128).T)
             for b in range(B)]
    return shared, xts, kbias


def kernel(x, attn_mask, Wq, bq, Wk, bk, Wv, bv, Wo, bo):
    runA, colA, runB, colB = _get_runners()
    shared, xts, kbias = _host_inputs(
        x, attn_mask, Wq, bq, Wk, bk, Wv, bv, Wo, bo)

    mapsA = [dict(shared, xt=np.ascontiguousarray(xts[b][:, :CUT]),
                  kbias=kbias[b]) for b in range(B)]
    mapsB = [dict(shared, xt=xts[b], kbias=kbias[b]) for b in range(B)]

    outA = runA(mapsA)
    outB = runB(mapsB)
    jax.block_until_ready(outA)
    jax.block_until_ready(outB)
    resA = colA(outA, B)
    resB = colB(outB, B)

    out = np.empty((B, L, D_MODEL), np.float32)
    for b in range(B):
        out[b, :CUT] = resA[b]["out"]
        out[b, CUT:] = resB[b]["out"]
    return out
